# revision 4
# baseline (speedup 1.0000x reference)
"""Trainium2 Bass kernel for masked causal multi-head self-attention.

Problem shapes (hardcoded): B=2, T=2048, D=1024, H=16, DH=64.

Sharding: 8 cores, tensor-parallel over (batch, head-group):
core c -> batch b = c // 4, head group g = c % 4 (heads 4g..4g+3,
feature slice 256g..256g+256). Each core computes a partial [D, T]
(transposed) output for its batch; the host sums the 4 partials per
batch and transposes back.

v2: all matmul inputs in bfloat16 (fp32 matmul is half-rate on the PE),
weights pre-transposed on the host so every load DMA is contiguous,
diagonal score tiles trimmed to their valid column range, softmax
normalization reads the PSUM accumulator directly (no evacuation
copies), and the output partials are stored bf16 (host sums in f32).

Device algorithm per core (all matmuls bf16 in / f32 PSUM accumulate):
  - per q-tile n: Q^T/K^T chains [256, 512]; V chains for k-tiles
    4n..4n+3 (rows masked by data_mask, ones column DH carries the
    softmax denominator through the AV matmul); then attention for
    q-tile j=n over k-tiles 0..4n+3: scores^T = K^T x Q^T per head
    (two 64-partition matmuls run concurrently in PE row groups), exp
    on ScalarE (scale=1/8, no max subtraction; scores bounded ~8.2),
    causal masking only on the 128x128 diagonal block, AV accumulates
    o'^T [65, 2, 512] per head pair; normalization r = dm_q/(sum+eps)
    computed from PSUM row 64, broadcast on GPSIMD, applied on DVE;
    out^T partial = Wp_c^T @ o (bf16, summed on host).
"""

import numpy as np

B, T, D, H = 2, 2048, 1024, 16
DH = D // H          # 64
HPC = 4              # heads per core
DC = HPC * DH        # 256 feature slice per core
NC = 8               # cores
QT = 512             # q tile width
KT = 128             # k tile width (partition dim)
NQT = T // QT        # 4
NKT = T // KT        # 16
SCALE = float(DH) ** -0.5

_cached = {}


MM_DTYPE = "bfloat16"  # "float32r" (accurate) or "bfloat16" (fast)


def _build_program(mm_dtype=None):
    import concourse.tile as tile
    from concourse import bacc, mybir

    F32 = mybir.dt.float32
    MDT = getattr(mybir.dt, mm_dtype or MM_DTYPE)
    ODT = F32 if MDT == mybir.dt.float32r else MDT
    EXP = mybir.ActivationFunctionType.Exp

    nc = bacc.Bacc("TRN2", target_bir_lowering=False, debug=False)

    xT_d = nc.dram_tensor("xT", [D, T], MDT, kind="ExternalInput")
    wq_d = nc.dram_tensor("wq", [128, D // 128, DC], MDT, kind="ExternalInput")
    wk_d = nc.dram_tensor("wk", [128, D // 128, DC], MDT, kind="ExternalInput")
    wv_d = nc.dram_tensor("wv", [128, D // 128, DC], MDT, kind="ExternalInput")
    wp_d = nc.dram_tensor("wp", [128, DC // 128, D], MDT, kind="ExternalInput")
    dm01_d = nc.dram_tensor("dm01", [KT, NKT], F32, kind="ExternalInput")
    dmr_d = nc.dram_tensor("dmrow2", [1, NQT, 2, QT], F32, kind="ExternalInput")
    tri_d = nc.dram_tensor("tri", [KT, KT], MDT, kind="ExternalInput")
    out_d = nc.dram_tensor("outT", [D, T], ODT, kind="ExternalOutput")

    with tile.TileContext(nc) as tc:
        with (
            tc.tile_pool(name="w", bufs=1) as wpool,
            tc.tile_pool(name="acts", bufs=1) as acts,
            tc.tile_pool(name="wt", bufs=4) as wtp,
            tc.tile_pool(name="sm", bufs=2) as sm,
            tc.tile_pool(name="ob", bufs=2) as obp,
            tc.tile_pool(name="psA", bufs=2, space="PSUM") as psA,
            tc.tile_pool(name="psS", bufs=2, space="PSUM") as psS,
            tc.tile_pool(name="psO", bufs=1, space="PSUM") as psO,
        ):
            # ---- loads (ordered by first use) ----
            wq = wpool.tile([128, D // 128, DC], MDT)
            nc.sync.dma_start(out=wq[:], in_=wq_d[:])
            xTs = []
            for kt in range(D // 128):
                c = wpool.tile([128, T], MDT, tag=f"xt{kt}")
                nc.sync.dma_start(out=c[:], in_=xT_d[128 * kt:128 * kt + 128, :])
                xTs.append(c)
            wk = wpool.tile([128, D // 128, DC], MDT)
            nc.sync.dma_start(out=wk[:], in_=wk_d[:])
            wv = wpool.tile([128, D // 128, DC], MDT)
            nc.sync.dma_start(out=wv[:], in_=wv_d[:])
            dm01 = wpool.tile([KT, NKT], F32)
            nc.sync.dma_start(out=dm01[:], in_=dm01_d[:])
            tri = wpool.tile([KT, KT], MDT)
            nc.sync.dma_start(out=tri[:], in_=tri_d[:])
            dmr = wpool.tile([1, NQT, 2, QT], F32)
            nc.sync.dma_start(out=dmr[:], in_=dmr_d[:])
            wp = wpool.tile([128, DC // 128, D], MDT)
            nc.sync.dma_start(out=wp[:], in_=wp_d[:])
            ones4 = wpool.tile([128, HPC], F32)
            nc.vector.memset(ones4[:], 1.0)

            qTn, kTn, vpt = [], [], []
            for n in range(NQT):
                tq = acts.tile([128, 2, QT], MDT, tag=f"qt{n}")
                tk = acts.tile([128, 2, QT], MDT, tag=f"kt{n}")
                qTn.append(tq)
                kTn.append(tk)
            for t in range(NKT):
                tv = acts.tile([128, HPC, DH + 1], MDT, tag=f"vp{t}")
                vpt.append(tv)

            for n in range(NQT):
                # ---- Q^T / K^T projections for q/k tile n ----
                for dsts, w in ((qTn, wq), (kTn, wk)):
                    for m in range(2):
                        ps = psA.tile([128, QT], F32, tag="pa")
                        for kt in range(D // 128):
                            nc.tensor.matmul(
                                ps[:],
                                w[:, kt, 128 * m:128 * m + 128],
                                xTs[kt][:, QT * n:QT * n + QT],
                                start=(kt == 0), stop=(kt == D // 128 - 1),
                            )
                        if m == 0:
                            nc.scalar.copy(dsts[n][:, m, :], ps[:])
                        else:
                            nc.vector.tensor_copy(dsts[n][:, m, :], ps[:])

                # ---- V (masked, with ones column) for k tiles 4n..4n+3 ----
                for t in range(4 * n, 4 * n + 4):
                    ps = psA.tile([128, DC], F32, tag="pa")
                    for kt in range(D // 128):
                        nc.tensor.matmul(
                            ps[:],
                            xTs[kt][:, 128 * t:128 * t + 128],
                            wv[:, kt, :],
                            start=(kt == 0), stop=(kt == D // 128 - 1),
                        )
                    nc.vector.tensor_scalar_mul(
                        vpt[t][:, :, 0:DH],
                        ps[:].rearrange("p (h d) -> p h d", h=HPC),
                        dm01[:, t:t + 1],
                    )
                    nc.vector.tensor_scalar_mul(
                        vpt[t][:, :, DH], ones4[:], dm01[:, t:t + 1],
                    )

                # ---- attention for q tile j = n ----
                j = n
                o_all = sm.tile([128, 2, QT], MDT, tag="oall")
                for m in range(2):  # head pairs (2m, 2m+1)
                    nkt = 4 * j + 4  # causal: k tiles 0 .. 4j+3
                    o_ps = psO.tile([DH + 1, 2, QT], F32, tag="ops")
                    for i in range(nkt):
                        r = i - 4 * j
                        w0 = 128 * r if r > 0 else 0
                        ps_s = psS.tile([128, 2, QT], F32, tag="ps")
                        for u in range(2):
                            p0 = 64 * u
                            nc.tensor.matmul(
                                ps_s[:, u, w0:QT],
                                kTn[i // 4][p0:p0 + 64, m,
                                            128 * (i % 4):128 * (i % 4) + 128],
                                qTn[j][p0:p0 + 64, m, w0:QT],
                                start=True, stop=True,
                            )
                        wt = wtp.tile([128, 2, QT], MDT, tag="wt")
                        nc.scalar.activation(
                            wt[:, :, w0:QT], ps_s[:, :, w0:QT], EXP,
                            bias=0.0, scale=SCALE)
                        for u in range(2):
                            if r >= 0:  # causal mask: 128x128 diagonal block
                                nc.vector.tensor_mul(
                                    wt[:, u, w0:w0 + KT],
                                    wt[:, u, w0:w0 + KT],
                                    tri[:],
                                )
                            nc.tensor.matmul(
                                o_ps[:, u, w0:QT],
                                vpt[i][:, 2 * m + u, :],
                                wt[:, u, w0:QT],
                                start=(i == 0), stop=(i == nkt - 1),
                            )
                    # normalization: r = dm_q / (sums + eps); sums ride
                    # PSUM partition 64 (ones column of V)
                    r0 = sm.tile([1, 2, QT], F32, tag="r0")
                    nc.vector.tensor_scalar_add(
                        r0[:], o_ps[DH:DH + 1, :, :], 1e-30)
                    rf = sm.tile([1, 2, QT], F32, tag="rf")
                    nc.vector.reciprocal_approx_fast(out=rf[:], in_=r0[:])
                    r2 = sm.tile([1, 2, QT], F32, tag="r2")
                    nc.vector.tensor_mul(r2[:], rf[:], dmr[:, j, :, :])
                    rb = sm.tile([64, 2, QT], F32, tag="rb")
                    nc.gpsimd.partition_broadcast(rb[:], r2[:], channels=64)
                    for u in range(2):
                        nc.vector.tensor_mul(
                            o_all[64 * u:64 * u + 64, m, :],
                            o_ps[0:DH, u, :], rb[:, u, :],
                        )
                # ---- output projection for q tile j ----
                for dt in range(D // 128):
                    pp = psA.tile([128, QT], F32, tag="pa")
                    for kt in range(2):
                        nc.tensor.matmul(
                            pp[:],
                            wp[:, kt, 128 * dt:128 * dt + 128],
                            o_all[:, kt, :],
                            start=(kt == 0), stop=(kt == 1),
                        )
                    ob = obp.tile([128, QT], ODT, tag="ob")
                    if dt % 2 == 0:
                        nc.vector.tensor_copy(ob[:], pp[:])
                    else:
                        nc.scalar.copy(ob[:], pp[:])
                    nc.sync.dma_start(
                        out=out_d[128 * dt:128 * dt + 128, QT * j:QT * j + QT],
                        in_=ob[:],
                    )

    nc.finalize()
    return nc


def _make_in_maps(x, data_mask, Wq, Wk, Wv, Wp, mm_dtype=None):
    if (mm_dtype or MM_DTYPE) == "bfloat16":
        import ml_dtypes
        mdt = ml_dtypes.bfloat16
    else:
        mdt = np.float32
    x = np.ascontiguousarray(np.asarray(x, np.float32))
    dm = np.asarray(data_mask).astype(np.float32)
    p = np.arange(KT)[:, None]
    q = np.arange(KT)[None, :]
    tri = (q >= p).astype(np.float32).astype(mdt)

    def wsplit(W, sl, colslice):
        W = np.asarray(W, np.float32)
        Wc = W[:, sl] if colslice else W[sl, :]
        nb = Wc.shape[0] // 128
        return np.ascontiguousarray(
            Wc.reshape(nb, 128, Wc.shape[1]).transpose(1, 0, 2).astype(mdt))

    in_maps = []
    for c in range(NC):
        b, g = divmod(c, HPC)
        sl = slice(DC * g, DC * g + DC)
        dmb = dm[b]
        dmr = np.repeat(dmb.reshape(NQT, 1, QT), 2, axis=1)[None]
        in_maps.append({
            "xT": np.ascontiguousarray(x[b].T.astype(mdt)),
            "wq": wsplit(Wq, sl, True),
            "wk": wsplit(Wk, sl, True),
            "wv": wsplit(Wv, sl, True),
            "wp": wsplit(Wp, sl, False),
            "dm01": np.ascontiguousarray(dmb.reshape(NKT, KT).T),
            "dmrow2": np.ascontiguousarray(dmr),
            "tri": tri,
        })
    return in_maps


def _postprocess(results, data_mask, bp):
    out = np.empty((B, T, D), np.float32)
    for b in range(B):
        acc = results[HPC * b]["outT"].astype(np.float32)
        for g in range(1, HPC):
            acc = acc + results[HPC * b + g]["outT"].astype(np.float32)
        out[b] = acc.T
    bp = np.asarray(bp, np.float32)
    if np.any(bp):
        # general path: device skipped bp and the final row mask folding
        # assumes bp == 0, so apply both here
        out = (out + bp) * np.asarray(data_mask, np.float32)[..., None]
    return out


def _numpy_reference(x, data_mask, Wq, bq, Wk, bk, Wv, bv, Wp, bp):
    # general fallback (only used when q/k/v biases are nonzero, which
    # does not happen for this problem's setup_inputs)
    x = np.asarray(x, np.float64)
    dm = np.asarray(data_mask) != 0
    q = (x @ np.asarray(Wq, np.float64) + np.asarray(bq, np.float64))
    k = (x @ np.asarray(Wk, np.float64) + np.asarray(bk, np.float64))
    v = (x @ np.asarray(Wv, np.float64) + np.asarray(bv, np.float64))
    q = q.reshape(B, T, H, DH).transpose(0, 2, 1, 3) * SCALE
    k = k.reshape(B, T, H, DH).transpose(0, 2, 1, 3)
    v = v.reshape(B, T, H, DH).transpose(0, 2, 1, 3)
    causal = np.tril(np.ones((T, T), bool))
    out = np.empty((B, T, D), np.float64)
    for b in range(B):
        mask = causal & dm[b][:, None] & dm[b][None, :]
        for h in range(H):
            s = q[b, h] @ k[b, h].T
            s = np.where(mask, s, -np.inf)
            s -= np.max(s, axis=-1, keepdims=True)
            w = np.exp(s)
            denom = w.sum(-1, keepdims=True)
            w = np.where(denom > 0, w / np.where(denom == 0, 1, denom), 0.0)
            w = np.nan_to_num(w)
            out[b, :, h * DH:(h + 1) * DH] = w @ v[b, h]
    out = out @ np.asarray(Wp, np.float64) + np.asarray(bp, np.float64)
    out *= dm[..., None]
    return out.astype(np.float32)


def kernel(x, data_mask, Wq, bq, Wk, bk, Wv, bv, Wp, bp):
    if any(np.any(np.asarray(v)) for v in (bq, bk, bv)):
        return _numpy_reference(x, data_mask, Wq, bq, Wk, bk, Wv, bv, Wp, bp)

    from concourse.bass_utils import run_bass_kernel_spmd

    if "nc" not in _cached:
        _cached["nc"] = _build_program()
    nc = _cached["nc"]
    in_maps = _make_in_maps(x, data_mask, Wq, Wk, Wv, Wp)
    res = run_bass_kernel_spmd(nc, in_maps, core_ids=list(range(NC)))
    return _postprocess(res.results, data_mask, bp)
